# revision 16
# baseline (speedup 1.0000x reference)
"""MinLSTM fused kernel for TRN2 (8 NeuronCores, batch-parallel), bf16.

Math (equivalent to the reference's log-space form):
    zf = x@Wf+bf ; zi = x@Wi+bi ; zh = x@Wh+bh
    Ef = exp(-zf)               # 1/sigmoid(zf) = 1 + Ef
    si = sigmoid(zi) ; sh = sigmoid(zh)
    g  = max(zh + bh + 0.5, sh)
    p  = (1 + Ef) * si          # = si/sf
    S  = 0.5 + cumsum(p*g, axis=time)
    out[:, 0, :]  = 0.5
    out[:, t+1, :] = S[t] / (1 + p[t])

Per core (one batch element): bf16 GEMMs z^T [512h, 4096t] (stationary =
W chunk, moving = x^T chunk, 4 rotating PSUM slots of [128,1024]).
Elementwise in [H-partition, T-free]: ACT does Exp/Sigmoid/Reciprocal in
table-batched phases per h-chunk pair (exp -> sig -> recip); DVE does the
g/p STTs and the fp32-accum scan (bf16 out); Pool does the u and o
tensor muls (bf16). Output written bf16, host transposes + upconverts.
"""
import numpy as np

_CACHE = {}

B, T, D, H = 8, 4096, 512, 512
NCORES = 8
N_HC = H // 128       # 4 h-chunks
N_D = D // 128        # 4 contraction chunks
N_TC = 4              # 1024-wide T chunks per h for GEMM+ACT
TCW = 1024
HALF = 2048           # p/u/scan/r2/o granularity
XW_COLS = 3 * H + T   # 5632


def _install_tilefix():
    """This walrus build accepts only ONE sync wait per hardware instruction;
    Tile can emit several. Spill extras onto injected single-wait drains."""
    import concourse.tile as tile
    from concourse import mybir
    from concourse.vector_clock import ScopedClock

    if getattr(tile.TileContext, "_minlstm_patched", False):
        return
    orig_lower = tile.TileContext._lower_ordered_insts

    def _spill_waits(self, ordered):
        nc = self.nc
        for bb_name, insts in ordered.items():
            out = []
            for inst in insts:
                si = inst.sync_info
                if si is not None and len(si.on_wait) > 1 and inst.engine is not None:
                    waits = list(si.on_wait)
                    for w in waits[:-1]:
                        d = mybir.InstDrain(
                            name=nc.get_next_instruction_name(),
                            ins=[], outs=[], bass_is_fusable=False,
                            sync_info=mybir.SyncInfo(on_wait=[w], on_update=[]),
                        )
                        d.engine = inst.engine
                        out.append(d)
                    si.on_wait = [waits[-1]]
                out.append(inst)
            insts[:] = out
        return ordered

    def _patched_lower(self, ordered):
        return orig_lower(self, _spill_waits(self, ordered))

    def _split_drain_and_barrier(self, tick_clock, wait_clock):
        drain_inst = self.nc.sync.drain()
        wait_clock.add_sem_waits(
            drain_inst.ins, ScopedClock({None: tick_clock.global_clock})
        )
        si = drain_inst.ins.sync_info
        if si is not None and len(si.on_wait) > 1:
            waits = list(si.on_wait)
            si.on_wait = [waits[0]]
            for w in waits[1:]:
                extra = self.nc.sync.drain()
                esi = extra.ins.sync_info
                if esi is None:
                    extra.ins.sync_info = mybir.SyncInfo(on_wait=[w], on_update=[])
                else:
                    esi.on_wait = [w]
        self.nc.all_engine_barrier()
        assert self.sems is not None
        popped = self.nc._tile_sem_poison_stack.pop()
        assert popped is self._sem_poison
        self.nc.clear_and_free_semaphores(list(self.sems.allocated().values()))
        self.nc.all_engine_barrier()

    tile.TileContext._lower_ordered_insts = _patched_lower
    tile.TileContext._drain_and_barrier = _split_drain_and_barrier
    tile.TileContext._minlstm_patched = True


def _build():
    import concourse.bass as bass
    import concourse.tile as tile
    from concourse import mybir
    from concourse.tile_rust import add_dep_helper

    _install_tilefix()

    f32 = mybir.dt.float32
    bf16 = mybir.dt.bfloat16
    AF = mybir.ActivationFunctionType
    ALU = mybir.AluOpType

    nc = bass.Bass("TRN2", target_bir_lowering=False, debug=False,
                   num_devices=NCORES)

    xw_d = nc.dram_tensor("xw", [D, XW_COLS], bf16, kind="ExternalInput").ap()
    bias_d = nc.dram_tensor("biases", [128, 16], f32, kind="ExternalInput").ap()
    out_d = nc.dram_tensor("out", [H, T], bf16, kind="ExternalOutput").ap()

    prev_act = [None]

    def act_raw(out, in_, func, bias=0.0, scale=1.0):
        eng = nc.scalar
        inputs = [eng.lower_ap(in_)]
        for arg in (bias, scale, 0.0):
            if isinstance(arg, bass.AP):
                inputs.append(eng.lower_ap(arg))
            else:
                inputs.append(
                    mybir.ImmediateValue(dtype=f32, value=float(arg))
                )
        i = eng.add_instruction(
            mybir.InstActivation(
                name=nc.get_next_instruction_name(),
                func=func, ins=inputs, outs=[eng.lower_ap(out)],
            )
        )
        if prev_act[0] is not None:
            add_dep_helper(i.ins, prev_act[0].ins, sync=False,
                           reason="ACT table-set order")
        prev_act[0] = i
        return i

    with tile.TileContext(nc) as tc:
        with (
            tc.tile_pool(name="xwp", bufs=1) as xwp,
            tc.tile_pool(name="cons", bufs=1) as cons,
            tc.tile_pool(name="ps", bufs=2, space="PSUM") as ps,
            tc.tile_pool(name="grid", bufs=2) as grid,
            tc.tile_pool(name="shp", bufs=4) as shp,
            tc.tile_pool(name="pp", bufs=4) as pp,
            tc.tile_pool(name="up", bufs=2) as up,
            tc.tile_pool(name="Sp", bufs=4) as Sp,
            tc.tile_pool(name="S1kp", bufs=8) as S1kp,
            tc.tile_pool(name="rp", bufs=4) as rp,
            tc.tile_pool(name="op", bufs=1) as op,
            tc.tile_pool(name="o1kp", bufs=2) as o1kp,
        ):
            xw = [
                xwp.tile([128, XW_COLS], bf16, tag=f"xw{d}", name=f"xw{d}")
                for d in range(N_D)
            ]
            # Wf first, then x^T tc0, then Wi/Wh, then remaining x^T chunks
            for d in range(N_D):
                nc.sync.dma_start(
                    xw[d][:, 0:H], xw_d[128 * d:128 * (d + 1), 0:H])
            for d in range(N_D):
                c0 = 3 * H
                nc.sync.dma_start(
                    xw[d][:, c0:c0 + HALF],
                    xw_d[128 * d:128 * (d + 1), c0:c0 + HALF])
            for d in range(N_D):
                nc.sync.dma_start(
                    xw[d][:, H:3 * H], xw_d[128 * d:128 * (d + 1), H:3 * H])
            for d in range(N_D):
                c0 = 3 * H + HALF
                nc.sync.dma_start(
                    xw[d][:, c0:],
                    xw_d[128 * d:128 * (d + 1), c0:])
            bt = cons.tile([128, 16], f32, tag="bt")
            nc.sync.dma_start(bt[:], bias_d[:])
            zero1 = cons.tile([128, 8], f32, tag="zero1")
            nc.vector.memset(zero1[:], 0.0)
            zb2k = zero1[:, 0:1].broadcast_to([128, HALF])
            zb1k = zero1[:, 0:1].broadcast_to([128, TCW])

            # PSUM: 2 x [128,2048] groups (4 banks each); 16 matmuls fill a
            # group, one 2048-wide ACT read (or 2x1024 piece reads) drains it.
            def gemm2(gate, h, tc2, name):
                z = ps.tile([128, HALF], f32, tag="z", name=name)
                for q in range(4):
                    sl = slice(512 * q, 512 * (q + 1))
                    t0 = 3 * H + HALF * tc2 + 512 * q
                    for d in range(N_D):
                        nc.tensor.matmul(
                            z[:, sl], xw[d][:, 512 * gate + 128 * h:
                                            512 * gate + 128 * h + 128],
                            xw[d][:, t0:t0 + 512],
                            start=(d == 0), stop=(d == N_D - 1),
                        )
                return z

            prev_q = {"v": None, "g": None}

            def vch(bi):
                if prev_q["v"] is not None:
                    add_dep_helper(bi.ins, prev_q["v"].ins, sync=False,
                                   reason="dve order")
                prev_q["v"] = bi
                return bi

            def gch(bi):
                if prev_q["g"] is not None:
                    add_dep_helper(bi.ins, prev_q["g"].ins, sync=False,
                                   reason="pool order")
                prev_q["g"] = bi
                return bi

            St = {}
            pt = {}
            rt = {}
            gt = {}
            ut = {}
            post = []   # deferred Pool outs from previous pair

            for pair in range(2):
                hs = (2 * pair, 2 * pair + 1)
                last = pair == 1

                Ef = {}
                si = {}
                g = {}
                for h in hs:
                    Ef[h] = grid.tile([128, T], bf16, tag="Ef", name=f"Ef{h}")
                    si[h] = grid.tile([128, T], bf16, tag="si", name=f"si{h}")
                    g[h] = grid.tile([128, T], bf16, tag="g", name=f"g{h}")

                # ---- EXP ----
                for h in hs:
                    nbf_ap = bt[:, h:h + 1]
                    for tc2 in range(2):
                        z = gemm2(0, h, tc2, f"zf{h}_{tc2}")
                        act_raw(Ef[h][:, HALF * tc2:HALF * (tc2 + 1)],
                                z[:], AF.Exp, bias=nbf_ap, scale=-1.0)

                # previous pair's Pool outs ride under this pair's exp/sigA
                for emit in post:
                    emit()
                post = []

                # ---- SIG-A: zi -> si ; p ----
                for h in hs:
                    bi_ap = bt[:, 4 + h:5 + h]
                    pt[h] = []
                    for tc2 in range(2):
                        z = gemm2(1, h, tc2, f"zi{h}_{tc2}")
                        act_raw(si[h][:, HALF * tc2:HALF * (tc2 + 1)],
                                z[:], AF.Sigmoid, bias=bi_ap)
                for h in hs:
                    for tc2 in range(2):
                        p = pp.tile([128, HALF], bf16, tag="p",
                                    name=f"p{h}_{tc2}")
                        vch(nc.vector.scalar_tensor_tensor(
                            out=p[:], in0=Ef[h][:, HALF * tc2:HALF * (tc2 + 1)],
                            scalar=1.0,
                            in1=si[h][:, HALF * tc2:HALF * (tc2 + 1)],
                            op0=ALU.add, op1=ALU.mult,
                        ))
                        pt[h].append(p)

                # ---- RECIP-A: r2 for h_even (both h's on the last pair) ----
                ra = hs if last else (hs[0],)
                for h in ra:
                    rt[h] = []
                    for tc2 in range(2):
                        r2 = rp.tile([128, HALF], bf16, tag="r2",
                                     name=f"r2_{h}_{tc2}")
                        act_raw(r2[:], pt[h][tc2][:], AF.Reciprocal, bias=1.0)
                        rt[h].append(r2)

                # ---- SIG-B: zh -> sh, g (u on Pool; scans chained after) ----
                for h in hs:
                    bg_ap = bt[:, 8 + h:9 + h]
                    bh_ap = bt[:, 12 + h:13 + h]
                    St[h] = []
                    gt[h] = []
                    ut[h] = []
                    lasth = last and h == hs[1]
                    if not lasth:
                        for tc2 in range(2):
                            z = gemm2(2, h, tc2, f"zh{h}_{tc2}")
                            sh = shp.tile([128, HALF], bf16, tag="sh",
                                          name=f"sh{h}_{tc2}")
                            act_raw(sh[:], z[:], AF.Sigmoid, bias=bh_ap)
                            vch(nc.vector.scalar_tensor_tensor(
                                out=g[h][:, HALF * tc2:HALF * (tc2 + 1)],
                                in0=z[:], scalar=bg_ap, in1=sh[:],
                                op0=ALU.add, op1=ALU.max,
                            ))
                            u = up.tile([128, HALF], bf16, tag="u",
                                        name=f"u{h}_{tc2}")
                            gch(nc.gpsimd.tensor_tensor(
                                out=u[:], in0=pt[h][tc2][:],
                                in1=g[h][:, HALF * tc2:HALF * (tc2 + 1)],
                                op=ALU.mult))
                            ut[h].append(u)
                    else:
                        # final h: @1024 pieces, whole chain on DVE
                        for tcol in range(N_TC):
                            if tcol % 2 == 0:
                                zg = gemm2(2, h, tcol // 2,
                                           f"zh{h}_{tcol // 2}")
                            zp = zg[:, TCW * (tcol % 2):TCW * (tcol % 2 + 1)]
                            sh = shp.tile([128, TCW], bf16, tag="sh1k",
                                          name=f"sh1k{tcol}")
                            act_raw(sh[:], zp, AF.Sigmoid, bias=bh_ap)
                            vch(nc.vector.scalar_tensor_tensor(
                                out=g[h][:, TCW * tcol:TCW * (tcol + 1)],
                                in0=zp, scalar=bg_ap, in1=sh[:],
                                op0=ALU.add, op1=ALU.max,
                            ))
                            u = up.tile([128, TCW], bf16, tag="u1k",
                                        name=f"u1k{tcol}")
                            vch(nc.vector.tensor_tensor(
                                out=u[:],
                                in0=pt[h][tcol // 2][:, TCW * (tcol % 2):
                                                     TCW * (tcol % 2 + 1)],
                                in1=g[h][:, TCW * tcol:TCW * (tcol + 1)],
                                op=ALU.mult))
                            S = S1kp.tile([128, TCW], bf16, tag="S1k",
                                          name=f"S1k{tcol}")
                            init = (0.5 if tcol == 0
                                    else St[h][-1][:, TCW - 1:TCW])
                            vch(nc.vector.tensor_tensor_scan(
                                S[:], zb1k, u[:], init, ALU.add, ALU.add))
                            St[h].append(S)
                            o = o1kp.tile([128, TCW], bf16, tag="o1k",
                                          name=f"o1k{tcol}")
                            vch(nc.vector.tensor_tensor(
                                out=o[:], in0=S[:],
                                in1=rt[h][tcol // 2][:, TCW * (tcol % 2):
                                                     TCW * (tcol % 2 + 1)],
                                op=ALU.mult))
                            nc.sync.dma_start(
                                out_d[128 * h:128 * (h + 1),
                                      TCW * tcol:TCW * (tcol + 1)],
                                o[:],
                            )

                # bulk scans (DVE) after the pair's g's; they spill into the
                # next pair's exp window harmlessly
                for h in hs:
                    if last and h == hs[1]:
                        continue
                    Sprev = None
                    for tc2 in range(2):
                        S = Sp.tile([128, HALF], bf16, tag="S",
                                    name=f"S{h}_{tc2}")
                        init = 0.5 if tc2 == 0 else Sprev[:, HALF - 1:HALF]
                        vch(nc.vector.tensor_tensor_scan(
                            S[:], zb2k, ut[h][tc2][:], init, ALU.add, ALU.add))
                        Sprev = S
                        St[h].append(S)

                # ---- RECIP-B: r2 for h_odd (non-last pairs), at boundary ----
                if not last:
                    h = hs[1]
                    rt[h] = []
                    for tc2 in range(2):
                        r2 = rp.tile([128, HALF], bf16, tag="r2",
                                     name=f"r2_{h}_{tc2}")
                        act_raw(r2[:], pt[h][tc2][:], AF.Reciprocal, bias=1.0)
                        rt[h].append(r2)

                # Pool outs: last pair's h_even inline; others deferred to
                # the next pair's exp window
                def mk_out(h):
                    def emit():
                        for tc2 in range(2):
                            o = op.tile([128, HALF], bf16, tag="o",
                                        name=f"o{h}_{tc2}")
                            gch(nc.gpsimd.tensor_tensor(
                                out=o[:], in0=St[h][tc2][:],
                                in1=rt[h][tc2][:], op=ALU.mult))
                            nc.sync.dma_start(
                                out_d[128 * h:128 * (h + 1),
                                      HALF * tc2:HALF * (tc2 + 1)],
                                o[:],
                            )
                    return emit

                if last:
                    mk_out(hs[0])()
                else:
                    post.append(mk_out(hs[0]))
                    post.append(mk_out(hs[1]))
            for emit in post:
                emit()
    return nc


def _get_nc():
    if "nc" not in _CACHE:
        _CACHE["nc"] = _build()
    return _CACHE["nc"]


def _make_in_maps(x, Wf, bf, Wi, bi, Wh, bh):
    import ml_dtypes
    bft = ml_dtypes.bfloat16

    x = np.asarray(x, dtype=np.float32)
    W_all = np.concatenate(
        [np.asarray(Wf), np.asarray(Wi), np.asarray(Wh)], axis=1
    ).astype(bft)

    bf32 = np.asarray(bf, dtype=np.float32)
    bi32 = np.asarray(bi, dtype=np.float32)
    bh32 = np.asarray(bh, dtype=np.float32)
    biases = np.zeros((128, 16), dtype=np.float32)
    biases[:, 0:4] = (-bf32).reshape(N_HC, 128).T
    biases[:, 4:8] = bi32.reshape(N_HC, 128).T
    biases[:, 8:12] = (bh32 + np.float32(0.5)).reshape(N_HC, 128).T
    biases[:, 12:16] = bh32.reshape(N_HC, 128).T

    in_maps = []
    for c in range(NCORES):
        xT = np.ascontiguousarray(x[c].T).astype(bft)
        xw = np.concatenate([W_all, xT], axis=1)
        in_maps.append({"xw": xw, "biases": biases})
    return in_maps


def kernel(x, Wf, bf, Wi, bi, Wh, bh):
    from concourse.bass_utils import run_bass_kernel_spmd

    in_maps = _make_in_maps(x, Wf, bf, Wi, bi, Wh, bh)
    nc = _get_nc()
    res = run_bass_kernel_spmd(nc, in_maps, list(range(NCORES)))

    out = np.empty((B, T + 1, H), dtype=np.float32)
    out[:, 0, :] = np.float32(0.5)
    for c in range(NCORES):
        out[c, 1:, :] = np.asarray(res.results[c]["out"]).astype(np.float32).T
    return out


# revision 17
# speedup vs baseline: 1.2502x; 1.2502x over previous
"""MinLSTM fused kernel for TRN2 (8 NeuronCores, batch-parallel), bf16.

Math (equivalent to the reference's log-space form):
    zf = x@Wf+bf ; zi = x@Wi+bi ; zh = x@Wh+bh
    Ef = exp(-zf)               # 1/sigmoid(zf) = 1 + Ef
    si = sigmoid(zi) ; sh = sigmoid(zh)
    g  = max(zh + bh + 0.5, sh)
    p  = (1 + Ef) * si          # = si/sf
    S  = 0.5 + cumsum(p*g, axis=time)
    out[:, 0, :]  = 0.5
    out[:, t+1, :] = S[t] / (1 + p[t])

Per core (one batch element): bf16 GEMMs z^T [512h, 4096t] (stationary =
W chunk, moving = x^T chunk, 4 rotating PSUM slots of [128,1024]).
Elementwise in [H-partition, T-free]: ACT does Exp/Sigmoid/Reciprocal in
table-batched phases per h-chunk pair (exp -> sig -> recip); DVE does the
g/p STTs and the fp32-accum scan (bf16 out); Pool does the u and o
tensor muls (bf16). Output written bf16, host transposes + upconverts.
"""
import numpy as np

_CACHE = {}

B, T, D, H = 8, 4096, 512, 512
NCORES = 8
N_HC = H // 128       # 4 h-chunks
N_D = D // 128        # 4 contraction chunks
N_TC = 4              # 1024-wide T chunks per h for GEMM+ACT
TCW = 1024
HALF = 2048           # p/u/scan/r2/o granularity
XW_COLS = 3 * H + T   # 5632


def _install_tilefix():
    """This walrus build accepts only ONE sync wait per hardware instruction;
    Tile can emit several. Spill extras onto injected single-wait drains."""
    import concourse.tile as tile
    from concourse import mybir
    from concourse.vector_clock import ScopedClock

    if getattr(tile.TileContext, "_minlstm_patched", False):
        return
    orig_lower = tile.TileContext._lower_ordered_insts

    def _spill_waits(self, ordered):
        nc = self.nc
        for bb_name, insts in ordered.items():
            out = []
            for inst in insts:
                si = inst.sync_info
                if si is not None and len(si.on_wait) > 1 and inst.engine is not None:
                    waits = list(si.on_wait)
                    for w in waits[:-1]:
                        d = mybir.InstDrain(
                            name=nc.get_next_instruction_name(),
                            ins=[], outs=[], bass_is_fusable=False,
                            sync_info=mybir.SyncInfo(on_wait=[w], on_update=[]),
                        )
                        d.engine = inst.engine
                        out.append(d)
                    si.on_wait = [waits[-1]]
                out.append(inst)
            insts[:] = out
        return ordered

    def _patched_lower(self, ordered):
        return orig_lower(self, _spill_waits(self, ordered))

    def _split_drain_and_barrier(self, tick_clock, wait_clock):
        drain_inst = self.nc.sync.drain()
        wait_clock.add_sem_waits(
            drain_inst.ins, ScopedClock({None: tick_clock.global_clock})
        )
        si = drain_inst.ins.sync_info
        if si is not None and len(si.on_wait) > 1:
            waits = list(si.on_wait)
            si.on_wait = [waits[0]]
            for w in waits[1:]:
                extra = self.nc.sync.drain()
                esi = extra.ins.sync_info
                if esi is None:
                    extra.ins.sync_info = mybir.SyncInfo(on_wait=[w], on_update=[])
                else:
                    esi.on_wait = [w]
        self.nc.all_engine_barrier()
        assert self.sems is not None
        popped = self.nc._tile_sem_poison_stack.pop()
        assert popped is self._sem_poison
        self.nc.clear_and_free_semaphores(list(self.sems.allocated().values()))
        self.nc.all_engine_barrier()

    tile.TileContext._lower_ordered_insts = _patched_lower
    tile.TileContext._drain_and_barrier = _split_drain_and_barrier
    tile.TileContext._minlstm_patched = True


def _build():
    import concourse.bass as bass
    import concourse.tile as tile
    from concourse import mybir
    from concourse.tile_rust import add_dep_helper

    _install_tilefix()

    f32 = mybir.dt.float32
    bf16 = mybir.dt.bfloat16
    AF = mybir.ActivationFunctionType
    ALU = mybir.AluOpType

    nc = bass.Bass("TRN2", target_bir_lowering=False, debug=False,
                   num_devices=NCORES)

    xw_d = nc.dram_tensor("xw", [D, XW_COLS], bf16, kind="ExternalInput").ap()
    bias_d = nc.dram_tensor("biases", [128, 16], f32, kind="ExternalInput").ap()
    out_d = nc.dram_tensor("out", [H, T], bf16, kind="ExternalOutput").ap()

    prev_act = [None]

    def act_raw(out, in_, func, bias=0.0, scale=1.0):
        eng = nc.scalar
        inputs = [eng.lower_ap(in_)]
        for arg in (bias, scale, 0.0):
            if isinstance(arg, bass.AP):
                inputs.append(eng.lower_ap(arg))
            else:
                inputs.append(
                    mybir.ImmediateValue(dtype=f32, value=float(arg))
                )
        i = eng.add_instruction(
            mybir.InstActivation(
                name=nc.get_next_instruction_name(),
                func=func, ins=inputs, outs=[eng.lower_ap(out)],
            )
        )
        if prev_act[0] is not None:
            add_dep_helper(i.ins, prev_act[0].ins, sync=False,
                           reason="ACT table-set order")
        prev_act[0] = i
        return i

    with tile.TileContext(nc) as tc:
        with (
            tc.tile_pool(name="xwp", bufs=1) as xwp,
            tc.tile_pool(name="cons", bufs=1) as cons,
            tc.tile_pool(name="ps", bufs=4, space="PSUM") as ps,
            tc.tile_pool(name="grid", bufs=2) as grid,
            tc.tile_pool(name="shp", bufs=4) as shp,
            tc.tile_pool(name="pp", bufs=4) as pp,
            tc.tile_pool(name="up", bufs=4) as up,
            tc.tile_pool(name="Sp", bufs=4) as Sp,
            tc.tile_pool(name="S1kp", bufs=8) as S1kp,
            tc.tile_pool(name="rp", bufs=4) as rp,
            tc.tile_pool(name="op", bufs=4) as op,
        ):
            xw = [
                xwp.tile([128, XW_COLS], bf16, tag=f"xw{d}", name=f"xw{d}")
                for d in range(N_D)
            ]
            # Wf first, then x^T tc0, then Wi/Wh, then remaining x^T chunks
            for d in range(N_D):
                nc.sync.dma_start(
                    xw[d][:, 0:H], xw_d[128 * d:128 * (d + 1), 0:H])
            for d in range(N_D):
                c0 = 3 * H
                nc.sync.dma_start(
                    xw[d][:, c0:c0 + TCW],
                    xw_d[128 * d:128 * (d + 1), c0:c0 + TCW])
            for d in range(N_D):
                nc.sync.dma_start(
                    xw[d][:, H:3 * H], xw_d[128 * d:128 * (d + 1), H:3 * H])
            for tcol in range(1, N_TC):
                c0 = 3 * H + TCW * tcol
                for d in range(N_D):
                    nc.sync.dma_start(
                        xw[d][:, c0:c0 + TCW],
                        xw_d[128 * d:128 * (d + 1), c0:c0 + TCW])
            bt = cons.tile([128, 16], f32, tag="bt")
            nc.sync.dma_start(bt[:], bias_d[:])
            zero1 = cons.tile([128, 8], f32, tag="zero1")
            nc.vector.memset(zero1[:], 0.0)
            zb = zero1[:, 0:1].broadcast_to([128, HALF])
            zb1k = zero1[:, 0:1].broadcast_to([128, TCW])

            def gemm(gate, h, tcol, name):
                z = ps.tile([128, TCW], f32, tag="z", name=name)
                for half in range(2):
                    sl = slice(512 * half, 512 * (half + 1))
                    t0 = 3 * H + TCW * tcol + 512 * half
                    for d in range(N_D):
                        nc.tensor.matmul(
                            z[:, sl], xw[d][:, 512 * gate + 128 * h:
                                            512 * gate + 128 * h + 128],
                            xw[d][:, t0:t0 + 512],
                            start=(d == 0), stop=(d == N_D - 1),
                        )
                return z

            backlog = []

            def drain(n):
                for _ in range(min(n, len(backlog))):
                    backlog.pop(0)()

            for pair in range(2):
                hs = (2 * pair, 2 * pair + 1)
                last = pair == 1

                Ef = {}
                si = {}
                g = {}
                for h in hs:
                    Ef[h] = grid.tile([128, T], bf16, tag="Ef", name=f"Ef{h}")
                    si[h] = grid.tile([128, T], bf16, tag="si", name=f"si{h}")
                    g[h] = grid.tile([128, T], bf16, tag="g", name=f"g{h}")

                # ---- EXP phase: zf GEMMs -> Ef = exp(-zf) ----
                for h in hs:
                    nbf_ap = bt[:, h:h + 1]            # -bf
                    for tcol in range(N_TC):
                        z = gemm(0, h, tcol, f"zf{h}_{tcol}")
                        act_raw(Ef[h][:, TCW * tcol:TCW * (tcol + 1)], z[:],
                                AF.Exp, bias=nbf_ap, scale=-1.0)
                        if tcol % 2 == 1:
                            drain(1)    # prev-pair scan/out piece

                # ---- SIG phase: zi -> si ; zh -> sh, g ----
                pt = {h: [] for h in hs}
                ut = {h: [] for h in hs}
                St = {h: [] for h in hs}
                for h in hs:
                    bi_ap = bt[:, 4 + h:5 + h]
                    bg_ap = bt[:, 8 + h:9 + h]         # bh + 0.5
                    bh_ap = bt[:, 12 + h:13 + h]
                    for tcol in range(N_TC):
                        zi = gemm(1, h, tcol, f"zi{h}_{tcol}")
                        act_raw(si[h][:, TCW * tcol:TCW * (tcol + 1)], zi[:],
                                AF.Sigmoid, bias=bi_ap)
                        zh = gemm(2, h, tcol, f"zh{h}_{tcol}")
                        sh = shp.tile([128, TCW], bf16, tag="sh",
                                      name=f"sh{h}_{tcol}")
                        act_raw(sh[:], zh[:], AF.Sigmoid, bias=bh_ap)
                        nc.vector.scalar_tensor_tensor(
                            out=g[h][:, TCW * tcol:TCW * (tcol + 1)],
                            in0=zh[:], scalar=bg_ap, in1=sh[:],
                            op0=ALU.add, op1=ALU.max,
                        )
                        if tcol % 2 == 1:
                            drain(1)    # prev-pair scan/out piece

                # ---- DVE p-STTs (all first: unblocks r2 + next exp) ----
                for h in hs:
                    for half in range(2):
                        sl = slice(HALF * half, HALF * (half + 1))
                        p = pp.tile([128, HALF], bf16, tag="p",
                                    name=f"p{h}_{half}")
                        nc.vector.scalar_tensor_tensor(
                            out=p[:], in0=Ef[h][:, sl], scalar=1.0,
                            in1=si[h][:, sl], op0=ALU.add, op1=ALU.mult,
                        )
                        pt[h].append(p)
                # ---- u on DVE (bf16 TT; self-contained chain) ----
                for h in hs:
                    for k in range(N_TC):
                        sl = slice(TCW * k, TCW * (k + 1))
                        u = up.tile([128, TCW], bf16, tag="u1k",
                                    name=f"u1k{h}_{k}")
                        nc.vector.tensor_tensor(
                            out=u[:],
                            in0=pt[h][k // 2][:, TCW * (k % 2):TCW * (k % 2 + 1)],
                            in1=g[h][:, sl], op=ALU.mult)
                        ut[h].append(u)
                # ---- ACT r2 @2048 (recip table) ----
                rt = {h: [] for h in hs}
                for h in hs:
                    for half in range(2):
                        r2 = rp.tile([128, HALF], bf16, tag="r2",
                                     name=f"r2_{h}_{half}")
                        act_raw(r2[:], pt[h][half][:], AF.Reciprocal,
                                bias=1.0)
                        rt[h].append(r2)

                # ---- queue scan + out pieces (drained during next pair) ----
                Sprev = {h: None for h in hs}

                def mk_piece(h, k, ut=ut, rt=rt, Sprev=Sprev, last=last):
                    def emit():
                        S = S1kp.tile([128, TCW], bf16, tag="S1k",
                                      name=f"S{h}_{k}")
                        init = 0.5 if k == 0 else Sprev[h][:, TCW - 1:TCW]
                        nc.vector.tensor_tensor_scan(
                            S[:], zb1k, ut[h][k][:], init, ALU.add, ALU.add
                        )
                        Sprev[h] = S
                        o = op.tile([128, TCW], bf16, tag="o",
                                    name=f"o{h}_{k}")
                        r2sl = rt[h][k // 2][:, TCW * (k % 2):TCW * (k % 2 + 1)]
                        if last:
                            nc.vector.tensor_tensor(out=o[:], in0=S[:],
                                                    in1=r2sl, op=ALU.mult)
                        else:
                            nc.gpsimd.tensor_tensor(out=o[:], in0=S[:],
                                                    in1=r2sl, op=ALU.mult)
                        nc.sync.dma_start(
                            out_d[128 * h:128 * (h + 1),
                                  TCW * k:TCW * (k + 1)],
                            o[:],
                        )
                    return emit

                for h in hs:
                    for k in range(N_TC):
                        backlog.append(mk_piece(h, k))
            drain(len(backlog))
    return nc


def _get_nc():
    if "nc" not in _CACHE:
        _CACHE["nc"] = _build()
    return _CACHE["nc"]


def _make_in_maps(x, Wf, bf, Wi, bi, Wh, bh):
    import ml_dtypes
    bft = ml_dtypes.bfloat16

    x = np.asarray(x, dtype=np.float32)
    W_all = np.concatenate(
        [np.asarray(Wf), np.asarray(Wi), np.asarray(Wh)], axis=1
    ).astype(bft)

    bf32 = np.asarray(bf, dtype=np.float32)
    bi32 = np.asarray(bi, dtype=np.float32)
    bh32 = np.asarray(bh, dtype=np.float32)
    biases = np.zeros((128, 16), dtype=np.float32)
    biases[:, 0:4] = (-bf32).reshape(N_HC, 128).T
    biases[:, 4:8] = bi32.reshape(N_HC, 128).T
    biases[:, 8:12] = (bh32 + np.float32(0.5)).reshape(N_HC, 128).T
    biases[:, 12:16] = bh32.reshape(N_HC, 128).T

    in_maps = []
    for c in range(NCORES):
        xT = np.ascontiguousarray(x[c].T).astype(bft)
        xw = np.concatenate([W_all, xT], axis=1)
        in_maps.append({"xw": xw, "biases": biases})
    return in_maps


def kernel(x, Wf, bf, Wi, bi, Wh, bh):
    from concourse.bass_utils import run_bass_kernel_spmd

    in_maps = _make_in_maps(x, Wf, bf, Wi, bi, Wh, bh)
    nc = _get_nc()
    res = run_bass_kernel_spmd(nc, in_maps, list(range(NCORES)))

    out = np.empty((B, T + 1, H), dtype=np.float32)
    out[:, 0, :] = np.float32(0.5)
    for c in range(NCORES):
        out[c, 1:, :] = np.asarray(res.results[c]["out"]).astype(np.float32).T
    return out
